# revision 46
# baseline (speedup 1.0000x reference)
"""Trainium2 Bass kernel for nn_ATTHScore (hyperbolic attention KNN scoring).

Self-contained: shards the full inputs across 8 NeuronCores (2 chunks of
1024 rows per core - pure data parallel), runs a Bass/Tile kernel per core,
gathers the full (16, 1024, 1024) score tensor.

Math (algebraically identical to the reference):
  With t = tanh(sqrt(c)*||v||), xv = <x, v/||v||>, A = c*||x||^2, c2 = 1-A:
    den = 1 - 2*sqrt(c)*t*xv + A*t^2
    u   = clip(c2*(1-t^2)/den, umin, 1)        # = 1 - artanh_arg^2
    artanh(arg) = ln(1+sqrt(1-u)) - ln(u)/2,  sqrt(1-u) = exp(0.5*ln(1-u))
    score = -(4/c)*artanh^2 + head_bias + tail_bias

v6 design, 283us -> 218us (engine busy: DVE ~173us, ACT ~155us, Pool ~80us):
  - table regions per chunk: [lnexp: A phase] [sig: tanh tiles + psum copies]
    [sqrt: B sqrt + next chunk's squares] [lnexp: B lns + next A's sheets].
    Square/Identity work in EVERY set, so squares and psum->SBUF copies act
    as region fillers for the serial ACT chain.
  - next chunk's a_pre (DMA + pair-norms + ln/exp + Pool normalize) batched
    BEFORE the tanh region so DVE has product/dot work during the
    tanh+copies ACT block.
  - tanh tiles stay f32 (bf16 tanh breaks the 2e-2 gate - verified);
    matmul x/v operands bf16 (verified: no accuracy change).
  - psum->SBUF copies pair-merged (two row-tiles per ACT copy).
  - tail cast to bf16 on the host (verified error-free): halves the largest
    input DMA stream and the vt SBUF; rot combines + tail normalize on DVE,
    ref combines on Pool; weights batched per chunk; tail_bias add on Pool
    (last tiles on DVE to shorten the drain); full-chunk sheet math; B(0)'s
    last ln/score tiles deferred into the B(1) window; chunk-1 a_pre spread
    through the A(0) loop; a mini tanh region inside LNEXP-2 lets the first
    B(1) den chains start early.
"""

import numpy as np
import ml_dtypes

import concourse.bacc as bacc
import concourse.mybir as mybir
import concourse.tile as tile
import concourse.dve_ops as dve_ops
from concourse import masks
from concourse.dve_spec import Spec, Src0, Src1, C0, C1, C2, One, sq, maxx, minn, lower
from concourse.dve_uop import DveOpSpec
from concourse.bass_utils import run_bass_kernel_spmd
from concourse.tile_rust import add_dep_helper
from contextlib import ExitStack

dt = mybir.dt
AF = mybir.ActivationFunctionType
ALU = mybir.AluOpType

# ---------------------------------------------------------------- constants
NCORES = 8
D = 512            # feature dim
CS = 1024          # chunk_size
NS = 1024          # neg_sample_size
NCHUNK = 16        # total chunks
CPC = NCHUNK // NCORES   # chunks per core = 2
BC = CPC * CS      # rows per core = 2048
NT = BC // 128     # row tiles per core = 16
TPC = CS // 128    # row tiles per chunk = 8
F32 = dt.float32
BF16 = dt.bfloat16
P = 128

BALL = float(np.float32(1.0 - 1e-5))
UMIN = float(np.float32(1.0 - np.float64(np.float32(1.0 - 1e-5)) ** 2))
MINN = 1e-15

# ------------------------------------------------------- custom DVE ops


def _register_op(name: str, spec: Spec) -> "dve_ops.DveOp":
    for existing in dve_ops.OPS:
        if existing.name == name:
            return existing
    shas = {}
    for ver in ("v3", "v4"):
        uops = lower(spec, ver=ver)
        shas[ver] = DveOpSpec(name=name, opcode=0, uops=uops, rd1_en=True).sha(ver)
    op = dve_ops.DveOp(name, spec, subdim=False, uops_sha=shas)
    dve_ops.OPS.append(op)
    dve_ops.CUSTOM_DVE_SPECS[name] = spec
    dve_ops._SUB_OPCODE_FOR_NAME[name] = max(dve_ops._SUB_OPCODE_FOR_NAME.values()) + 1
    assert dve_ops._SUB_OPCODE_FOR_NAME[name] < 0x20
    return op


# den = 1 + (mm*t)*C0 + t^2*C1     (C0 = -2*sqrt(c), C1 = A)
HYP_DEN = _register_op("HYP_DEN", Spec(
    body=(Src0 * Src1) * C0 + sq(Src1) * C1 + One,
    reference=lambda in0, in1, s0, s1, imm2: in0 * in1 * s0 + in1 * in1 * s1 + 1.0,
))
# u = min(max((1 - t^2)*C0*rden, C2), 1)     (C0 = c2, C2 = umin)
HYP_U = _register_op("HYP_U", Spec(
    body=minn(maxx(((One - sq(Src1)) * C0) * Src0, C2), One),
    reference=lambda in0, in1, s0, s1, imm2: np.minimum(
        np.maximum((1.0 - in1 * in1) * s0 * in0, imm2), 1.0),
))
# score-tb = (l1 - l2*C2)^2*C0 + C1          (C0 = -4/c, C1 = hb, C2 = 0.5)
HYP_SCORE = _register_op("HYP_SCORE", Spec(
    body=sq(Src0 - Src1 * C2) * C0 + C1,
    reference=lambda in0, in1, s0, s1, imm2: (in0 - in1 * imm2) ** 2 * s0 + s1,
))
# out = Src0*C0 + Src1*C1  (per-partition weighted sum)
HYP_WSUM = _register_op("HYP_WSUM", Spec(
    body=Src0 * C0 + Src1 * C1,
    reference=lambda in0, in1, s0, s1, imm2: in0 * s0 + in1 * s1,
))
# out = sq(Src0) + sq(Src1)  (pair norm^2)
HYP_N2 = _register_op("HYP_N2", Spec(
    body=sq(Src0) + sq(Src1),
    reference=lambda in0, in1, s0, s1, imm2: in0 * in0 + in1 * in1,
))
# tanh from exp: t = (E - 1) * rE1   (E pre-clamped; rE1 = 1/(E+1))
HYP_TFE = _register_op("HYP_TFE", Spec(
    body=(Src0 - One) * Src1,
    reference=lambda in0, in1, s0, s1, imm2: (in0 - 1.0) * in1,
))
# out = max(Src0 + Src1 + 1, C2)   (n1 with C2=-inf-ish, dm with C2=MINN)
HYP_ADD1M = _register_op("HYP_ADD1M", Spec(
    body=maxx(Src0 + Src1 + One, C2),
    reference=lambda in0, in1, s0, s1, imm2: np.maximum(in0 + in1 + 1.0, imm2),
))
# out = sq(Src0) * Src1   (a0^2*x2, b0^2*y2, g^2*rn2)
HYP_SQMUL = _register_op("HYP_SQMUL", Spec(
    body=sq(Src0) * Src1,
    reference=lambda in0, in1, s0, s1, imm2: in0 * in0 * in1,
))


# ---------------------------------------------------------------- kernel IR


def build_nc(debug: bool = False):
    nc = bacc.Bacc("TRN2", target_bir_lowering=False, debug=False,
                   num_devices=NCORES)

    head_in = nc.declare_dram_parameter("head", [BC, D], BF16, isOutput=False)
    hb_in = nc.declare_dram_parameter("head_bias", [BC, 1], F32, isOutput=False)
    rel_in = nc.declare_dram_parameter("rel", [BC, D], BF16, isOutput=False)
    rd_in = nc.declare_dram_parameter("rel_diag", [BC, 2 * D], BF16, isOutput=False)
    curv_in = nc.declare_dram_parameter("curvature", [BC, 1], F32, isOutput=False)
    ctx_in = nc.declare_dram_parameter("context", [BC, D], BF16, isOutput=False)
    scale_in = nc.declare_dram_parameter("scale", [1, 1], F32, isOutput=False)
    tail_in = nc.declare_dram_parameter("tail", [BC, D], F32, isOutput=False)
    tb_in = nc.declare_dram_parameter("tail_bias", [BC, 1], BF16, isOutput=False)
    score_out = nc.declare_dram_parameter("score", [BC, NS], BF16, isOutput=True)
    vn_scr = nc.dram_tensor("vn_scratch", [BC, 1], F32)

    def pairv(ap):
        return ap.rearrange("p (k two) -> p k two", two=2)

    with tile.TileContext(nc) as tc, ExitStack() as ctx:
        cpool = ctx.enter_context(tc.tile_pool(name="const", bufs=1))
        spool = ctx.enter_context(tc.tile_pool(name="sheets", bufs=1))
        apool = ctx.enter_context(tc.tile_pool(name="aflow", bufs=2))
        inpool = ctx.enter_context(tc.tile_pool(name="influx", bufs=4))
        n2pool = ctx.enter_context(tc.tile_pool(name="n2p", bufs=2))
        prpool = ctx.enter_context(tc.tile_pool(name="prhold", bufs=9))
        vtpool = ctx.enter_context(tc.tile_pool(name="vtiles", bufs=3))
        relpool = ctx.enter_context(tc.tile_pool(name="reltile", bufs=7))
        rlhold = ctx.enter_context(tc.tile_pool(name="rlhold", bufs=4))
        vnbpool = ctx.enter_context(tc.tile_pool(name="vnb", bufs=1))
        hold = ctx.enter_context(tc.tile_pool(name="hold", bufs=TPC))
        ttpool = ctx.enter_context(tc.tile_pool(name="ttiles", bufs=TPC - 2))
        dpool = ctx.enter_context(tc.tile_pool(name="dtiles", bufs=2))
        upool = ctx.enter_context(tc.tile_pool(name="utiles", bufs=TPC))
        lpool = ctx.enter_context(tc.tile_pool(name="ltiles", bufs=TPC))
        bpool = ctx.enter_context(tc.tile_pool(name="bflow", bufs=2))
        scrpool = ctx.enter_context(tc.tile_pool(name="scrp", bufs=1))
        vmpool = ctx.enter_context(tc.tile_pool(name="vmats", bufs=1))
        xmpool = ctx.enter_context(tc.tile_pool(name="xmats", bufs=1))
        pp_tp = ctx.enter_context(tc.tile_pool(name="ps_tp", bufs=2, space="PSUM"))
        pp_mm = ctx.enter_context(tc.tile_pool(name="ps_mm", bufs=3, space="PSUM"))

        identB_t = cpool.tile([P, P], BF16, name="identB")
        masks.make_identity(nc, identB_t[:])
        identB = identB_t[:]

        # ACT instruction chain: forces scalar-engine execution order so the
        # activation-table loads happen at planned set boundaries only.
        _last_act = [None]

        def ACT(*args, **kw):
            bi = nc.scalar.activation(*args, **kw)
            if _last_act[0] is not None:
                add_dep_helper(bi.ins, _last_act[0], sync=False,
                               reason="act-table-order")
            _last_act[0] = bi.ins
            return bi

        def ACT_COPY(dst, src):
            bi = nc.scalar.copy(dst, src)
            if _last_act[0] is not None:
                add_dep_helper(bi.ins, _last_act[0], sync=False,
                               reason="act-table-order")
            _last_act[0] = bi.ins
            return bi

        from concourse.hw_specs import get_activation_tables
        _tabs = list(get_activation_tables(nc.m.arch).keys())
        LNEXP_SET = _tabs.index("natural_log_exp_and_others")
        SIG_SET = _tabs.index("sigmoid_and_others")
        SQRT_SET = _tabs.index("sqrt_and_others")

        def ACT_LOAD(set_id):
            inst = mybir.InstLoadActFuncSet(
                name=nc.get_next_instruction_name(), act_func_set_id=set_id,
                ins=[], outs=[])
            bi = nc.scalar.add_instruction(inst)
            if _last_act[0] is not None:
                add_dep_helper(bi.ins, _last_act[0], sync=False,
                               reason="act-table-order")
            _last_act[0] = bi.ins
            return bi

        # ---- sheets: per-row scalars, col t = row-tile t, partition = row%128
        def sheet(nm, cols=NT):
            return spool.tile([P, cols], F32, tag=nm, name=nm)

        names = ("kcurv khb srot sref xy0 c_s sqc_s rsqc_s rc_s "
                 "m2sqc_s m4c_s A_s c2_s alpha_s beta_s "
                 "scr_s1 scr_s2 scr_s3 scr_s4 scr_s5 scr_s6 vn2_sh vn_sh "
                 "rvn_sh wrot_s wref_s wd_s").split()
        (kcurv, khb, srot, sref, xy0, c_s, sqc_s, rsqc_s, rc_s,
         m2sqc_s, m4c_s, A_s, c2_s, alpha_s, beta_s,
         scr_s1, scr_s2, scr_s3, scr_s4, scr_s5, scr_s6, vn2_sh, vn_sh,
         rvn_sh, wrot_s, wref_s, wd_s) = [sheet(nm) for nm in names]
        # paired sheets (128, 2*NT): [att-side | rel-side]
        norms2 = sheet("norms2", 2 * NT)   # [na2 | nr2]
        lnn2 = sheet("lnn2", 2 * NT)
        rnorm2 = sheet("rnorm2", 2 * NT)   # 1/norm
        zpair = sheet("zpair", 2 * NT)     # z = sqc*norm
        tpair = sheet("tpair", 2 * NT)     # tanh(z)
        fpair = sheet("fpair", 2 * NT)     # tanh(z)/z
        spair = sheet("spair", 2 * NT)     # tanh(z)/sqc
        qpair = sheet("qpair", 2 * NT)     # (tanh(z)/sqc)^2 = x2|y2

        def pair2(sh, cc, half=None):
            # (128, 2, n) strided view of a (128, 2*NT) paired sheet
            if half is None:
                c0, n = TPC * cc, TPC
            else:
                c0, n = TPC * cc + half * (TPC // 2), TPC // 2
            return sh[:].rearrange("p (two t) -> p two t", two=2)[
                :, :, c0:c0 + n]

        # prefill score rows with tail_bias; the score write-out then
        # accumulates on the DMA engine (drops the Pool add entirely)
        for cc in range(CPC):
            nc.sync.dma_start(
                score_out[cc * CS:(cc + 1) * CS, :],
                tb_in[cc * CS:(cc + 1) * CS, :].rearrange(
                    "(o n) one -> o (n one)", o=1).broadcast_to((CS, NS)))
        nc.sync.dma_start(kcurv[:], curv_in[:].rearrange("(t p) o -> p (t o)", p=P))
        nc.sync.dma_start(khb[:], hb_in[:].rearrange("(t p) o -> p (t o)", p=P))
        scale_bc = cpool.tile([P, 1], F32)
        nc.sync.dma_start(scale_bc[:], scale_in[:].broadcast_to((P, 1)))
        nscale_bc = cpool.tile([P, 1], F32)
        nc.vector.tensor_scalar(nscale_bc[:], scale_bc[:], -1.0, None, ALU.mult)

        ACT_LOAD(LNEXP_SET)
        # curvature chain (all 16 cols at once; natural_log_exp set)
        ACT(scr_s1[:], kcurv[:], AF.Exp)
        ACT(c_s[:], scr_s1[:], AF.Ln, bias=1.0)
        ACT(scr_s1[:], c_s[:], AF.Ln)
        ACT(sqc_s[:], scr_s1[:], AF.Exp, scale=0.5)
        ACT(rsqc_s[:], scr_s1[:], AF.Exp, scale=-0.5)
        nc.vector.tensor_tensor(rc_s[:], rsqc_s[:], rsqc_s[:], ALU.mult)
        nc.vector.tensor_scalar(m2sqc_s[:], sqc_s[:], -2.0, None, ALU.mult)
        nc.vector.tensor_scalar(m4c_s[:], rc_s[:], -4.0, None, ALU.mult)

        # ------------------------------------------------ state dicts
        pre_state = {}
        prod_state = {}
        att_state = {}
        vt_views = {}
        tt_tiles = {}
        u_tiles = {}
        l_tiles = {}
        vn_bs = {}
        tb_bs = {}
        vTs = {}
        xTs = {}

        # ------------------------------------------------ tail norms + bcasts
        def tail_load_pair(cc, q0):
            r0 = cc * CS + q0 * 128
            vt2 = vtpool.tile([P, 2 * D], F32, tag="vt2", name=f"vt2_{cc}_{q0}")
            nc.sync.dma_start(
                vt2[:].rearrange("p (b d) -> p b d", b=2),
                tail_in[r0:r0 + 256, :].rearrange("(b p) d -> p b d", p=P))
            vt_views[(cc, q0)] = vt2[:, 0:D]
            vt_views[(cc, q0 + 1)] = vt2[:, D:2 * D]

        def tail_sq(cc, q):
            col = TPC * cc + q
            scr = scrpool.tile([P, D], BF16, tag="sqscr", name=f"tsq{cc}_{q}")
            ACT(scr[:], vt_views[(cc, q)], AF.Square,
                accum_out=vn2_sh[:, col:col + 1])

        def tails_sheet(cc, half):
            h4 = TPC // 2
            sl = slice(TPC * cc + half * h4, TPC * cc + half * h4 + h4)
            # vn = exp(0.5 ln vn2)
            ACT(scr_s1[:, sl], vn2_sh[:, sl], AF.Ln)
            ACT(vn_sh[:, sl], scr_s1[:, sl], AF.Exp, scale=0.5)
            rows = vn_scr[cc * CS + half * h4 * P:cc * CS + (half + 1) * h4 * P, :]
            nc.scalar.dma_start(rows.rearrange("(t p) o -> p (t o)", p=P),
                                vn_sh[:, sl])

        def tails_bcast(cc):
            # DRAM scratch -> partition-broadcast load (ACT queue)
            scr_rows = vn_scr[cc * CS:(cc + 1) * CS, :]
            vn_b = vnbpool.tile([P, NS], F32, tag="vn_b", name=f"vnb{cc}")
            nc.scalar.dma_start(
                vn_b[:],
                scr_rows.rearrange("(o n) one -> o (n one)", o=1
                                   ).broadcast_to((P, NS)))
            vn_bs[cc] = vn_b
            # 1/vn sheet for the tail-normalize tensor_scalar
            sl = slice(TPC * cc, TPC * cc + TPC)
            nc.vector.reciprocal_approx_fast(rvn_sh[:, sl], vn_sh[:, sl])

        # ------------------------------------------------ tanh region
        def tanh_tile(cc, q):
            t = TPC * cc + q
            tt = ttpool.tile([P, NS], F32, tag="tt", name=f"tt{t}")
            ACT(tt[:], vn_bs[cc][:], AF.Tanh, scale=sqc_s[:, t:t + 1])
            tt_tiles[t] = tt

        # ------------------------------------------------ tail prep (vT)
        ptp_state = {}

        def bprep_pair(cc, q):
            # q even: normalize + transpose tiles q, q+1 into one double-ptp
            col = TPC * cc + q
            ptp = pp_tp.tile([P, 2 * D], BF16, tag="tp", name=f"vtp{cc}_{q}")
            vs_eng = nc.gpsimd if cc == 0 else nc.vector
            for k in (0, 1):
                vs = apool.tile([P, D], BF16, tag="vs", name=f"vs{cc}_{q+k}")
                vs_eng.tensor_scalar(vs[:], vt_views.pop((cc, q + k)),
                                     rvn_sh[:, col + k:col + k + 1], None,
                                     ALU.mult)
                for dk in range(4):
                    nc.tensor.transpose(
                        ptp[:, dk * 256 + k * 128:dk * 256 + (k + 1) * 128],
                        vs[:, dk * 128:(dk + 1) * 128],
                        identB)
            ptp_state[("v", cc, q)] = ptp

        def bprep_copy(cc, q):
            ptp = ptp_state.pop(("v", cc, q))
            vT = vTs[cc]
            ACT_COPY(
                vT[:].rearrange("p (dk n) -> p dk n", dk=4)[
                    :, :, q * 128:(q + 2) * 128],
                ptp[:].rearrange("p (dk n) -> p dk n", dk=4))

        # ------------------------------------------------ A phase per tile
        n2_state = {}

        hc_state = {}

        def a_pre_hc(cc, ti):
            # ti even: head/context/rel DMAs for tiles ti, ti+1
            t = TPC * cc + ti
            r0 = t * 128
            ht2 = inpool.tile([P, 2 * D], BF16, tag="h2", name=f"h2_{t}")
            ct2 = inpool.tile([P, 2 * D], BF16, tag="c2", name=f"c2_{t}")
            rlt2 = rlhold.tile([P, 2 * D], BF16, tag="rl2", name=f"rl2_{t}")
            nc.sync.dma_start(
                ht2[:].rearrange("p (b d) -> p b d", b=2),
                head_in[r0:r0 + 256, :].rearrange("(b p) d -> p b d", p=P))
            nc.sync.dma_start(
                ct2[:].rearrange("p (b d) -> p b d", b=2),
                ctx_in[r0:r0 + 256, :].rearrange("(b p) d -> p b d", p=P))
            nc.sync.dma_start(
                rlt2[:].rearrange("p (b d) -> p b d", b=2),
                rel_in[r0:r0 + 256, :].rearrange("(b p) d -> p b d", p=P))
            hc_state[(cc, ti)] = (ht2, ct2, rlt2)

        def a_pre_dma(cc, ti, hc=True):
            # ti even: rel_diag DMAs + pair-norms^2 for tiles ti, ti+1
            t = TPC * cc + ti
            if hc:
                a_pre_hc(cc, ti)
            rdt2a = relpool.tile([P, 2 * D], BF16, tag="rd1", name=f"rd_{t}")
            rdt2b = relpool.tile([P, 2 * D], BF16, tag="rd1", name=f"rd_{t+1}")
            nc.sync.dma_start(rdt2a[:], rd_in[t * 128:(t + 1) * 128, :])
            nc.sync.dma_start(rdt2b[:], rd_in[(t + 1) * 128:(t + 2) * 128, :])
            n2d = n2pool.tile([P, 2 * D], F32, tag="n2d", name=f"n2d{t}")
            nc.vector._custom_dve(HYP_N2, out=n2d[:, 0:D],
                                  in0=rdt2a[0:P, 0:2 * D:2],
                                  in1=rdt2a[0:P, 1:2 * D:2])
            nc.vector._custom_dve(HYP_N2, out=n2d[:, D:2 * D],
                                  in0=rdt2b[0:P, 0:2 * D:2],
                                  in1=rdt2b[0:P, 1:2 * D:2])
            n2_state[(cc, ti)] = (n2d, rdt2a, rdt2b)

        def a_pre_lnexp(cc, ti):
            # rsq = exp(-0.5 ln n2); pair-normalize rel_diag in place (Pool)
            t = TPC * cc + ti
            n2d, rdt2a, rdt2b = n2_state.pop((cc, ti))
            ACT(n2d[:], n2d[:], AF.Ln)
            rsqd = n2pool.tile([P, 2 * D], BF16, tag="rsqd", name=f"rsq{t}")
            ACT(rsqd[:], n2d[:], AF.Exp, scale=-0.5)
            for k, rdt in ((0, rdt2a), (1, rdt2b)):
                nc.gpsimd.tensor_tensor(
                    pairv(rdt[:]), pairv(rdt[:]),
                    rsqd[:, k * D:(k + 1) * D].unsqueeze(-1)
                    .broadcast_to((P, D, 2)), ALU.mult)
            pre_state[(cc, ti)] = (rdt2a[:],)
            pre_state[(cc, ti + 1)] = (rdt2b[:],)

        def m_prod(cc, ti):
            t = TPC * cc + ti
            if ti % 2 == 0 and (cc, ti) not in hc_state:
                a_pre_hc(cc, ti)
            ht2, ct2, rlt2 = hc_state[(cc, ti - ti % 2)]
            k = ti % 2
            ht = ht2[:, k * D:(k + 1) * D]
            ct = ct2[:, k * D:(k + 1) * D]
            rlt = rlt2[:, k * D:(k + 1) * D]
            if k == 1:
                hc_state.pop((cc, ti - 1))
            (rdt,) = pre_state.pop((cc, ti))
            # products: all on DVE in bf16 (2x mode)
            hsw = pairv(ht)[:, :, ::-1]
            pr = prpool.tile([P, D], BF16, tag="rot", name=f"pr{t}")
            qr = apool.tile([P, D], BF16, tag="qr", name=f"qr{t}")
            pf = prpool.tile([P, D], BF16, tag="ref", name=f"pf{t}")
            qf = apool.tile([P, D], BF16, tag="qf", name=f"qf{t}")
            nc.vector.tensor_tensor(pr[:], rdt[0:P, 0:D], ht, ALU.mult)
            nc.vector.tensor_tensor(pairv(qr[:]), pairv(rdt[0:P, 0:D]), hsw,
                                    ALU.mult)
            nc.vector.tensor_tensor(pf[:], rdt[0:P, D:2 * D], ht, ALU.mult)
            nc.vector.tensor_tensor(pairv(qf[:]), pairv(rdt[0:P, D:2 * D]), hsw,
                                    ALU.mult)
            # combines: rot on DVE in the A(0) window (Pool-bound there),
            # on Pool during the B(0)||A(1) window (DVE-bound there)
            rot_eng = nc.vector if cc == 0 else nc.gpsimd
            rot_eng.tensor_tensor(pr[:, 0:D:2], pr[:, 0:D:2], pr[:, 1:D:2],
                                  ALU.subtract)
            rot_eng.tensor_tensor(pr[:, 1:D:2], qr[:, 0:D:2], qr[:, 1:D:2],
                                  ALU.add)
            nc.gpsimd.tensor_tensor(pf[:, 0:D:2], pf[:, 0:D:2], pf[:, 1:D:2],
                                    ALU.add)
            nc.gpsimd.tensor_tensor(pf[:, 1:D:2], qf[:, 1:D:2], qf[:, 0:D:2],
                                    ALU.subtract)
            prod_state[t] = (pr, pf, ct, rlt)

        def m_dot(cc, ti):
            t = TPC * cc + ti
            pr, pf, ct, rlt = prod_state[t]
            scr = scrpool.tile([P, D], BF16, tag="scr", name=f"sc1_{t}")
            nc.vector.affine_mul_reduce(scr[:], srot[:, t:t + 1], ct, pr[:],
                                        1.0, 0.0)
            scr2 = scrpool.tile([P, D], BF16, tag="scr", name=f"sc2_{t}")
            nc.vector.affine_mul_reduce(scr2[:], sref[:, t:t + 1], ct, pf[:],
                                        1.0, 0.0)
            # rel norm^2 (independent of weights)
            scr6 = scrpool.tile([P, D], BF16, tag="scrA", name=f"sc6_{t}")
            ACT(scr6[:], rlt, AF.Square, accum_out=norms2[:, NT + t:NT + t + 1])

        def a_w(cc, lo, hi):
            # batched softmax weights for tiles [lo, hi) of chunk cc
            sl = slice(TPC * cc + lo, TPC * cc + hi)
            nc.vector.tensor_tensor(wd_s[:, sl], srot[:, sl], sref[:, sl],
                                    ALU.subtract)
            ACT(wd_s[:, sl], wd_s[:, sl], AF.Exp, scale=nscale_bc[:])
            nc.vector.tensor_scalar(wd_s[:, sl], wd_s[:, sl], 1.0, None,
                                    ALU.add)
            nc.vector.reciprocal_approx_fast(wrot_s[:, sl], wd_s[:, sl])
            nc.vector.tensor_scalar(wref_s[:, sl], wrot_s[:, sl], -1.0,
                                    1.0, ALU.mult, ALU.add)

        def a_att(cc, ti):
            t = TPC * cc + ti
            pr, pf, ct, rlt = prod_state.pop(t)
            att = hold.tile([P, D], BF16, tag="att", name=f"att{t}")
            nc.vector._custom_dve(HYP_WSUM, out=att[:], in0=pr[:], in1=pf[:],
                                  s0=wrot_s[:, t:t + 1], s1=wref_s[:, t:t + 1])
            att_state[t] = (att, rlt)
            scr4 = scrpool.tile([P, D], BF16, tag="scrA", name=f"sc4_{t}")
            ACT(scr4[:], att[:], AF.Square, accum_out=norms2[:, t:t + 1])
            scr5 = scrpool.tile([P, D], BF16, tag="scr", name=f"sc5_{t}")
            nc.vector.affine_mul_reduce(scr5[:], xy0[:, t:t + 1], att[:], rlt,
                                        1.0, 0.0)

        # ------------------------------------------------ sheet math
        def s2_sheets(cc, half=None):
            if half is None:
                c0, n = TPC * cc, TPC
            else:
                c0, n = TPC * cc + half * (TPC // 2), TPC // 2
            sl = slice(c0, c0 + n)
            TT = nc.vector.tensor_tensor
            TS = nc.vector.tensor_scalar
            STT = nc.vector.scalar_tensor_tensor

            # paired norm chain: norm = exp(.5 ln n2), 1/norm = exp(-.5 ln n2)
            ACT(pair2(lnn2, cc, half), pair2(norms2, cc, half), AF.Ln)
            ACT(pair2(rnorm2, cc, half), pair2(lnn2, cc, half), AF.Exp, scale=-0.5)
            ACT(pair2(zpair, cc, half), pair2(lnn2, cc, half), AF.Exp, scale=0.5)
            sqcb = sqc_s[:, sl].unsqueeze(1).broadcast_to((P, 2, n))
            rsqcb = rsqc_s[:, sl].unsqueeze(1).broadcast_to((P, 2, n))
            TT(pair2(zpair, cc, half), pair2(zpair, cc, half), sqcb, ALU.mult)
            # tanh via exp (stay in lnexp set): E = exp(2z); t = (E-1)/(E+1)
            ACT(pair2(lnn2, cc, half), pair2(zpair, cc, half), AF.Exp, scale=2.0)
            TS(pair2(lnn2, cc, half), pair2(lnn2, cc, half), 3.0e37, None, ALU.min)
            TS(pair2(zpair, cc, half), pair2(lnn2, cc, half), 1.0, None, ALU.add)
            nc.vector.reciprocal_approx_fast(pair2(norms2, cc, half),
                                             pair2(zpair, cc, half))
            nc.vector._custom_dve(HYP_TFE, out=pair2(tpair, cc, half),
                                  in0=pair2(lnn2, cc, half),
                                  in1=pair2(norms2, cc, half))
            # f = tanh(z)/z = tanh(z) * (1/norm) * (1/sqc)
            TT(pair2(fpair, cc, half), pair2(tpair, cc, half),
               pair2(rnorm2, cc, half), ALU.mult)
            TT(pair2(fpair, cc, half), pair2(fpair, cc, half), rsqcb, ALU.mult)
            # s = tanh(z)/sqc ; q = s^2  (x2 | y2)
            TT(pair2(spair, cc, half), pair2(tpair, cc, half), rsqcb, ALU.mult)
            TT(pair2(qpair, cc, half), pair2(spair, cc, half),
               pair2(spair, cc, half), ALU.mult)

            q2 = qpair[:].rearrange("p (two t) -> p two t", two=2)
            x2 = q2[:, 0, sl]
            y2 = q2[:, 1, sl]
            f2 = fpair[:].rearrange("p (two t) -> p two t", two=2)
            fa = f2[:, 0, sl]
            fr = f2[:, 1, sl]

            xy = scr_s1
            TT(xy[:, sl], fa, fr, ALU.mult)
            TT(xy[:, sl], xy[:, sl], xy0[:, sl], ALU.mult)

            cxy2, cy2, cx2 = scr_s2, scr_s3, scr_s4
            STT(cxy2[:, sl], xy[:, sl], 2.0, c_s[:, sl], ALU.mult, ALU.mult)
            TT(cy2[:, sl], c_s[:, sl], y2, ALU.mult)
            TT(cx2[:, sl], c_s[:, sl], x2, ALU.mult)
            ccx2y2 = scr_s5
            TT(ccx2y2[:, sl], cx2[:, sl], cy2[:, sl], ALU.mult)
            n1, dm = scr_s6, scr_s3  # cy2 consumed after n1
            nc.vector._custom_dve(HYP_ADD1M, out=n1[:, sl], in0=cxy2[:, sl],
                                  in1=cy2[:, sl], imm2=-3.0e38)
            nc.vector._custom_dve(HYP_ADD1M, out=dm[:, sl], in0=cxy2[:, sl],
                                  in1=ccx2y2[:, sl], imm2=MINN)
            n2c = scr_s2  # cxy2 dead
            TS(n2c[:, sl], cx2[:, sl], -1.0, 1.0, ALU.mult, ALU.add)
            rdm = scr_s5  # ccx2y2 dead
            nc.vector.reciprocal_approx_fast(rdm[:, sl], dm[:, sl])
            a0, b0 = scr_s3, scr_s4  # dm, cx2 dead
            TT(a0[:, sl], n1[:, sl], rdm[:, sl], ALU.mult)
            TT(b0[:, sl], n2c[:, sl], rdm[:, sl], ALU.mult)

            # rn2 = a0^2*x2 + b0^2*y2 + 2*a0*b0*xy
            p1, p2 = scr_s6, scr_s2  # n1, n2c dead
            nc.vector._custom_dve(HYP_SQMUL, out=p1[:, sl], in0=a0[:, sl], in1=x2)
            nc.vector._custom_dve(HYP_SQMUL, out=p2[:, sl], in0=b0[:, sl], in1=y2)
            rn2 = scr_s5  # rdm dead
            TT(rn2[:, sl], p1[:, sl], p2[:, sl], ALU.add)
            ab = scr_s6  # p1 dead
            TT(ab[:, sl], a0[:, sl], b0[:, sl], ALU.mult)
            TT(ab[:, sl], ab[:, sl], xy[:, sl], ALU.mult)
            STT(rn2[:, sl], ab[:, sl], 2.0, rn2[:, sl], ALU.mult, ALU.add)
            # g = min(maxn/rn, 1) with 1/rn = exp(-0.5 ln rn2)
            lnr = scr_s2
            ACT(lnr[:, sl], rn2[:, sl], AF.Ln)
            rrn = scr_s6
            ACT(rrn[:, sl], lnr[:, sl], AF.Exp, scale=-0.5)
            g = scr_s2
            STT(g[:, sl], rrn[:, sl], BALL, rsqc_s[:, sl], ALU.mult, ALU.mult)
            TS(g[:, sl], g[:, sl], 1.0, None, ALU.min)

            TT(alpha_s[:, sl], g[:, sl], a0[:, sl], ALU.mult)
            TT(alpha_s[:, sl], alpha_s[:, sl], fa, ALU.mult)
            TT(beta_s[:, sl], g[:, sl], b0[:, sl], ALU.mult)
            TT(beta_s[:, sl], beta_s[:, sl], fr, ALU.mult)
            gg = scr_s6  # rrn dead
            nc.vector._custom_dve(HYP_SQMUL, out=gg[:, sl], in0=g[:, sl],
                                  in1=rn2[:, sl])
            TT(A_s[:, sl], gg[:, sl], c_s[:, sl], ALU.mult)
            TS(c2_s[:, sl], A_s[:, sl], -1.0, 1.0, ALU.mult, ALU.add)

        # ------------------------------------------------ a3: res -> xT
        def a3_res(cc, ti):
            # ti even: res + transposes for tiles ti, ti+1 into one double-ptp
            ptp = pp_tp.tile([P, 2 * D], BF16, tag="tp", name=f"xtp{cc}_{ti}")
            for k in (0, 1):
                t = TPC * cc + ti + k
                att, rlt = att_state.pop(t)
                res = apool.tile([P, D], BF16, tag="res", name=f"res{t}")
                nc.vector._custom_dve(HYP_WSUM, out=res[:], in0=att[:],
                                      in1=rlt, s0=alpha_s[:, t:t + 1],
                                      s1=beta_s[:, t:t + 1])
                for dk in range(4):
                    nc.tensor.transpose(
                        ptp[:, dk * 256 + k * 128:dk * 256 + (k + 1) * 128],
                        res[:, dk * 128:(dk + 1) * 128],
                        identB)
            ptp_state[("x", cc, ti)] = ptp

        def a3_copy(cc, ti):
            ptp = ptp_state.pop(("x", cc, ti))
            dst = xTs[cc][:].rearrange("p (dk n) -> p dk n", dk=4)[
                :, :, ti * 128:(ti + 2) * 128]
            src = ptp[:].rearrange("p (dk n) -> p dk n", dk=4)
            ACT_COPY(dst, src)

        # ------------------------------------------------ B phase pieces
        def b1(cc, ti):
            t = TPC * cc + ti
            tcol = slice(t, t + 1)
            xT, vT = xTs[cc], vTs[cc]
            pmm = pp_mm.tile([P, NS], F32, tag="mm", name=f"mm{t}")
            for ns in range(2):
                for dk in range(4):
                    nc.tensor.matmul(
                        pmm[:, ns * 512:(ns + 1) * 512],
                        xT[:, dk * 1024 + ti * 128: dk * 1024 + (ti + 1) * 128],
                        vT[:, dk * 1024 + ns * 512: dk * 1024 + (ns + 1) * 512],
                        start=(dk == 0), stop=(dk == 3))
            tt = tt_tiles[t]
            den = dpool.tile([P, NS], F32, tag="bw1", name=f"den{t}")
            nc.vector._custom_dve(HYP_DEN, out=den[:], in0=pmm[:], in1=tt[:],
                                  s0=m2sqc_s[:, tcol], s1=A_s[:, tcol])
            nc.vector.reciprocal_approx_fast(den[:], den[:])
            u = upool.tile([P, NS], BF16, tag="bu", name=f"u{t}")
            nc.vector._custom_dve(HYP_U, out=u[:], in0=den[:], in1=tt[:],
                                  s0=c2_s[:, tcol], imm2=UMIN)
            u_tiles[t] = u

        s_tiles = {}

        def b_sqrt(cc, ti):
            # s = sqrt(1-u)  (sqrt set)
            t = TPC * cc + ti
            s_ = lpool.tile([P, NS], BF16, tag="bs", name=f"s{t}")
            ACT(s_[:], u_tiles[t][:], AF.Sqrt, bias=1.0, scale=-1.0)
            s_tiles[t] = s_

        def b_ln(cc, ti):
            # l1 = ln(1+s) over s; lu = ln(u) over u  (lnexp set, in place)
            t = TPC * cc + ti
            s_ = s_tiles.pop(t)
            ACT(s_[:], s_[:], AF.Ln, bias=1.0)
            u = u_tiles.pop(t)
            ACT(u[:], u[:], AF.Ln)
            l_tiles[t] = (s_, u)

        def b3(cc, ti, tb_dve=False):
            t = TPC * cc + ti
            tcol = slice(t, t + 1)
            l1, lu = l_tiles.pop(t)
            sc0 = lpool.tile([P, NS], BF16, tag="sc0", name=f"sc{t}")
            nc.vector._custom_dve(HYP_SCORE, out=sc0[:], in0=l1[:], in1=lu[:],
                                  s0=m4c_s[:, tcol], s1=khb[:, tcol], imm2=0.5)
            nc.gpsimd.dma_start(score_out[t * 128:(t + 1) * 128, :], sc0[:],
                                accum_op=ALU.add)

        # ================================================ emission
        # --- W1: A(0), all lnexp; staged pipeline to hide Pool latency ---
        a_pre_dma(0, 0)
        for q0 in (0, 2, 4, 6):
            tail_load_pair(0, q0)
        a_pre_dma(0, 2)
        for ti in range(TPC):
            if ti % 2 == 0:
                if ti >= 4:
                    a_pre_dma(0, ti)
                a_pre_lnexp(0, ti)
            if ti >= 1:
                m_prod(0, ti - 1)
            if ti >= 2:
                m_dot(0, ti - 2)
            if ti == 5:
                a_w(0, 0, 4)
                a_att(0, 0)
                a_att(0, 1)
            if ti == 6:
                a_att(0, 2)
                a_att(0, 3)
            tail_sq(0, ti)
            if ti == 3:
                tails_sheet(0, 0)
            if ti == 4:
                a_pre_dma(1, 0, hc=False)
                a_pre_lnexp(1, 0)
            if ti == 5:
                a_pre_dma(1, 2, hc=False)
                a_pre_lnexp(1, 2)
            if ti == 6:
                a_pre_dma(1, 4, hc=False)
                a_pre_lnexp(1, 4)
            if ti == 7:
                tails_sheet(0, 1)
                tails_bcast(0)
                a_pre_dma(1, 6, hc=False)
                a_pre_lnexp(1, 6)
        m_prod(0, 7)
        m_dot(0, 6)
        m_dot(0, 7)
        a_w(0, 4, 8)
        a_att(0, 4)
        a_att(0, 5)
        a_att(0, 6)
        a_att(0, 7)
        s2_sheets(0)
        vTs[0] = vmpool.tile([P, 4 * NS], BF16, tag="vT", name="vT0")
        for q0 in (0, 2, 4, 6):
            bprep_pair(0, q0)
        xTs[0] = xmpool.tile([P, 4 * NS], BF16, tag="xT", name="xT0")
        a3_res(0, 0)
        a3_res(0, 2)
        a3_res(0, 4)
        a3_res(0, 6)
        # chunk-1 inputs + pair-norms; rsqrt ln/exp still in this region,
        # interleaved with the chunk-0 psum copies (any-set ops)
        for q0 in (0, 2, 4, 6):
            tail_load_pair(1, q0)
        for k in (0, 2, 4, 6):
            a_pre_dma(1, k)
        a_pre_lnexp(1, 0)
        bprep_copy(0, 0)
        bprep_copy(0, 2)
        a_pre_lnexp(1, 2)
        bprep_copy(0, 4)
        bprep_copy(0, 6)
        a_pre_lnexp(1, 4)
        a3_copy(0, 0)
        a3_copy(0, 2)
        a_pre_lnexp(1, 6)
        a3_copy(0, 4)
        a3_copy(0, 6)

        # --- SIG-1: tanh(0); DVE runs den chain + chunk-1 products ---
        ACT_LOAD(SIG_SET)
        for q in range(TPC):
            tanh_tile(0, q)
        b1(0, 0)
        m_prod(1, 0)
        b1(0, 1)
        m_prod(1, 1)

        # --- SQRT-1: B(0) sqrt + squares as ACT fillers ---
        ACT_LOAD(SQRT_SET)
        for ti in range(TPC):
            if ti >= 2:
                b1(0, ti)
            b_sqrt(0, ti)
            if ti < 6:
                m_prod(1, ti + 2)
            if ti < 8:
                m_dot(1, ti)
            tail_sq(1, ti)

        # --- LNEXP-2: B(0) lns + scores; A(1) weights/sheets ---
        ACT_LOAD(LNEXP_SET)
        tails_sheet(1, 0)
        tails_sheet(1, 1)
        tails_bcast(1)
        a_w(1, 0, 8)
        for ti in range(TPC):
            if ti < 5:
                b_ln(0, ti)
            if ti < 4:
                a_att(1, 2 * ti)
                a_att(1, 2 * ti + 1)
            if ti < 5:
                b3(0, ti)
            if ti == 3:
                vTs[1] = vmpool.tile([P, 4 * NS], BF16, tag="vT",
                                     name="vT1")
                bprep_pair(1, 0)
                bprep_pair(1, 2)
            if ti == 4:
                s2_sheets(1)
                bprep_pair(1, 4)
                bprep_pair(1, 6)
            if ti == 5:
                xTs[1] = xmpool.tile([P, 4 * NS], BF16, tag="xT", name="xT1")
                a3_res(1, 0)
                a3_res(1, 2)
            if ti == 6:
                a3_res(1, 4)
            if ti == 7:
                a3_res(1, 6)
        # copies for chunk 1 (any-set) before the region switch
        bprep_copy(1, 0)
        bprep_copy(1, 2)
        bprep_copy(1, 4)
        bprep_copy(1, 6)
        a3_copy(1, 0)
        a3_copy(1, 2)
        a3_copy(1, 4)
        a3_copy(1, 6)

        # --- SIG-2: tanh(1); DVE starts den chain of B(1) ---
        ACT_LOAD(SIG_SET)
        for q in range(TPC):
            tanh_tile(1, q)
        b1(1, 0)
        b1(1, 1)

        # --- B(1): half-interleaved sqrt/ln regions ---
        ACT_LOAD(SQRT_SET)
        for ti in range(4):
            if ti >= 2:
                b1(1, ti)
            b_sqrt(1, ti)
        ACT_LOAD(LNEXP_SET)
        b_ln(0, 5)
        b3(0, 5)
        for ti in range(4):
            b1(1, ti + 4)
            b_ln(1, ti)
            b3(1, ti)
            if ti == 0:
                b_ln(0, 6)
                b3(0, 6)
            if ti == 1:
                b_ln(0, 7)
                b3(0, 7)
        ACT_LOAD(SQRT_SET)
        for ti in range(4, TPC):
            b_sqrt(1, ti)
        ACT_LOAD(LNEXP_SET)
        for ti in range(4, TPC):
            b_ln(1, ti)
            b3(1, ti, tb_dve=(ti >= 6))

    nc.finalize()
    return nc


_NC_CACHE = {}


def _get_nc(debug=False):
    if debug not in _NC_CACHE:
        _NC_CACHE[debug] = build_nc(debug)
    return _NC_CACHE[debug]


def kernel(head, head_bias, rel, rel_diag, curvature, context, scale, tail,
           tail_bias, chunk_size, neg_sample_size, _debug=False, _trace=False):
    cs = int(chunk_size)
    ns = int(neg_sample_size)
    assert cs == CS and ns == NS, (cs, ns)
    bf16 = ml_dtypes.bfloat16
    head = np.ascontiguousarray(np.asarray(head, np.float32).astype(bf16))
    rel_diag = np.ascontiguousarray(np.asarray(rel_diag, np.float32).astype(bf16))
    context = np.ascontiguousarray(np.asarray(context, np.float32).astype(bf16))
    head_bias = np.ascontiguousarray(np.asarray(head_bias, np.float32))
    rel = np.ascontiguousarray(np.asarray(rel, np.float32).astype(bf16))
    curvature = np.ascontiguousarray(np.asarray(curvature, np.float32))
    scale = np.ascontiguousarray(np.asarray(scale, np.float32)).reshape(1, 1)
    tail = np.ascontiguousarray(np.asarray(tail, np.float32))
    tail_bias = np.ascontiguousarray(np.asarray(tail_bias, np.float32).astype(bf16))

    nc = _get_nc(False)
    in_maps = []
    for core in range(NCORES):
        r = slice(core * BC, (core + 1) * BC)
        in_maps.append({
            "head": head[r], "head_bias": head_bias[r], "rel": rel[r],
            "rel_diag": rel_diag[r], "curvature": curvature[r],
            "context": context[r], "scale": scale, "tail": tail[r],
            "tail_bias": tail_bias[r],
        })
    res = run_bass_kernel_spmd(nc, in_maps, core_ids=list(range(NCORES)),
                               trace=_trace)
    score = np.concatenate(
        [np.asarray(res.results[c]["score"]).astype(np.float32)
         for c in range(NCORES)], axis=0)
    out = score.reshape(NCHUNK, CS, NS)
    if _trace:
        return out, res
    return out
